# revision 49
# baseline (speedup 1.0000x reference)
# Adaptive softmax (3-cluster) on 8 TRN2 NeuronCores.
#
# Strategy (moe_routing): each token only needs its own cluster's pipeline.
# Host-side we sort tokens by cluster (pure data movement), shard each
# cluster's token segment evenly over the 8 cores, and pad each per-core
# segment to a static capacity so the Bass graph stays shape-static.
# Per core, per cluster c with nt tokens / pd proj dim / C classes:
#   hiddenT[pd, nt] = p_c @ x_shard^T            (PE, bf16)
#   hidden  [nt, pd]                             (PE, bf16; token-major)
#   logits  [nt, C] = hidden @ w_c^T             (PE, fp8 DoubleRow, chunked)
#   sumexp  [nt]    = sum_C exp(logits)          (ScalarE exp -> DVE row-sum)
#   logit_t [nt]    = rowdot(hidden, w_c[tgt]) + b_c[tgt]    (DVE, bf16)
#   nll     [nt]    = ln(sumexp) - logit_t
# The big logits GEMM runs in fp8e4m3 DoubleRow (2 rows/PE cell = 2x MACs).
# fp8 range handling: weights are pre-scaled x64 and hidden x4 (powers of
# two, lossless), and the exp activation rescales with scale=1/256.
# No max-subtraction is needed: |logits| <~ 4 for this problem's scales.
# Classes are padded to uniform 512-wide chunks with zero weights; each
# padded class contributes exactly exp(0)=1, corrected via the Ln pre-bias.
# The target-row weights w_c[tgt] are gathered on host (data movement only).
# No collectives: pure data parallelism; host gathers/unpermutes/sums.

import numpy as np
import ml_dtypes
from contextlib import ExitStack

import concourse.bass as bass
import concourse.bacc as bacc
import concourse.mybir as mybir
import concourse.tile as tile
from concourse.bass_utils import run_bass_kernel_spmd

BF16 = mybir.dt.bfloat16
FP8 = mybir.dt.float8e4
F32 = mybir.dt.float32
bf16 = ml_dtypes.bfloat16
fp8 = ml_dtypes.float8_e4m3

VOCAB = 50257
D = 1024           # input dim
KD = D // 128      # k-tiles over input dim
CUT = [0, 10000, 30000, VOCAB]
PD = [1024, 512, 256]            # per-cluster projection dims
KP = [p // 128 for p in PD]      # k-tiles over proj dim
CSIZE = [CUT[i + 1] - CUT[i] for i in range(3)]
NCORES = 8
CAP = [256, 512, 512]            # per-core token capacity per cluster (padded)
TILES = [c // 128 for c in CAP]
SLOT0 = [0, 256, 768]            # slot offset of each cluster's segment
TOT = sum(CAP)                   # 1280 padded tokens per core
NT = TOT // 128                  # 10 token tiles per core
CHUNK = 512                      # class chunk (one PSUM bank of f32)

HSCALE = 4.0                     # hidden fp8 pre-scale (power of 2)
WSCALE = 64.0                    # weight fp8 pre-scale (power of 2)
EXP_SCALE = 1.0 / (HSCALE * WSCALE)

# class padding to uniform chunks
CPAD = [-(-C // CHUNK) * CHUNK for C in CSIZE]
NPAD = [CPAD[i] - CSIZE[i] for i in range(3)]
CHUNKS = [[(off, CHUNK) for off in range(0, CPAD[i], CHUNK)] for i in range(3)]

# cluster processing order: smallest input first (shrinks the pre-matmul
# head), c0 last (smallest tail)
ORDER = [2, 1, 0]

_GRAPH_CACHE = {}


def _build_graph():
    # Bacc (not plain Bass): its compile() pass splits semaphore waits into
    # event-semaphore carriers, satisfying TRN2's 1-wait-per-instruction limit.
    nc = bacc.Bacc(trn_type="TRN2", target_bir_lowering=False)

    xT_d = nc.dram_tensor("xT", [128, KD, TOT], BF16, kind="ExternalInput")
    pT_d = [
        nc.dram_tensor(f"pT{i}", [128, KD, PD[i]], BF16, kind="ExternalInput")
        for i in range(3)
    ]
    F = [KP[i] * CPAD[i] for i in range(3)]
    wT_d = [
        nc.dram_tensor(f"wT{i}", [128, F[i]], FP8, kind="ExternalInput")
        for i in range(3)
    ]
    # wg carries the gathered target-row weights plus one bias column
    wg_d = [
        nc.dram_tensor(f"wg{i}", [TILES[i], 128, PD[i] + 1], BF16, kind="ExternalInput")
        for i in range(3)
    ]
    out_d = nc.dram_tensor("out", [NT, 128], F32, kind="ExternalOutput")

    Exp = mybir.ActivationFunctionType.Exp
    Ln = mybir.ActivationFunctionType.Ln
    X = mybir.AxisListType.X
    DR = mybir.MatmulPerfMode.DoubleRow

    with ExitStack() as ctx:
        tc = ctx.enter_context(tile.TileContext(nc))
        const = ctx.enter_context(tc.tile_pool(name="const", bufs=1))
        wpool = ctx.enter_context(tc.tile_pool(name="wpool", bufs=8))
        hpool = ctx.enter_context(tc.tile_pool(name="hpool", bufs=1))
        spool = ctx.enter_context(tc.tile_pool(name="spool", bufs=2))
        tiny = ctx.enter_context(tc.tile_pool(name="tiny", bufs=1))
        psA = ctx.enter_context(tc.tile_pool(name="psA", bufs=2, space="PSUM"))
        psB = ctx.enter_context(tc.tile_pool(name="psB", bufs=3, space="PSUM"))

        # input DMAs for all clusters up front (processing order first) so a
        # cluster's pt/xt land before the previous cluster's weight stream
        # monopolizes the DMA lanes
        pts, xts = {}, {}
        for i in ORDER:
            pt = const.tile([128, KD * PD[i]], BF16, name=f"pt{i}")
            nc.gpsimd.dma_start(pt, pT_d[i][:, :, :])
            xt = const.tile([128, KD * CAP[i]], BF16, name=f"xt{i}")
            nc.gpsimd.dma_start(xt, xT_d[:, :, SLOT0[i]:SLOT0[i] + CAP[i]])
            pts[i], xts[i] = pt, xt

        for i in ORDER:
            ntok, t0, kp, pd, nti = CAP[i], SLOT0[i], KP[i], PD[i], TILES[i]
            ng = kp // 2  # DoubleRow contraction groups (K=256 each)

            pt3 = pts[i].rearrange("p (k m) -> p k m", k=KD)
            xt3 = xts[i].rearrange("p (k t) -> p k t", k=KD)

            # ---- Stage A1: hiddenT [pd, ntok] as fp8 (x4), pd-major ----
            hidT = hpool.tile([128, kp * ntok], FP8, name=f"hidT{i}")
            hidT3 = hidT.rearrange("p (k t) -> p k t", k=kp)
            hidT4 = hidT.rearrange("p (g j t) -> p g j t", g=ng, j=2)
            for mp in range(kp):
                ps = psA.tile([128, ntok], F32, name=f"psA1_{i}_{mp}", tag="psA")
                for k in range(KD):
                    nc.tensor.matmul(
                        ps,
                        lhsT=pt3[:, k, mp * 128:(mp + 1) * 128],
                        rhs=xt3[:, k, :],
                        start=(k == 0),
                        stop=(k == KD - 1),
                    )
                # f32 -> fp8 with x4 pre-scale, on ScalarE
                nc.scalar.mul(hidT3[:, mp, :], ps, HSCALE)

            # ---- Stage A2: token-major hidden (bf16) + target-logit dot ----
            nlts = []
            for ti in range(nti):
                ht = hpool.tile([128, pd + 1], BF16, name=f"ht{i}_{ti}")
                nc.vector.memset(ht[:, pd:pd + 1], 1.0)
                for c0 in range(0, pd, 512):
                    cw = min(512, pd - c0)
                    ps = psA.tile([128, cw], F32, name=f"psA2_{i}_{ti}_{c0}", tag="psA")
                    for k in range(KD):
                        nc.tensor.matmul(
                            ps,
                            lhsT=xt3[:, k, ti * 128:(ti + 1) * 128],
                            rhs=pt3[:, k, c0:c0 + cw],
                            start=(k == 0),
                            stop=(k == KD - 1),
                        )
                    nc.vector.tensor_copy(ht[:, c0:c0 + cw], ps)
                wgt = const.tile([128, pd + 1], BF16, name=f"wgt{i}_{ti}")
                nc.gpsimd.dma_start(wgt, wg_d[i][ti, :, :])
                # absorb the DMA wait on a TensorCopy so the TensorTensor
                # below needs at most one sem wait
                dm = tiny.tile([128, 1], BF16, name=f"dm{i}_{ti}")
                nc.vector.tensor_copy(dm, wgt[:, 0:1])
                prod = spool.tile([128, pd + 1], F32, name=f"prod{i}_{ti}", tag="prod")
                nlt = tiny.tile([128, 1], F32, name=f"nlt{i}_{ti}")
                nc.vector.tensor_mul(prod, ht, wgt)
                nc.vector.reduce_sum(nlt, prod, axis=X, negate=True)
                nlts.append(nlt)

            # ---- Stage B: fp8 DoubleRow logits + exp + running bf16 sum ----
            # chunks processed in pairs sharing one 2-bank PSUM tile; the exp
            # runs once per 1024 columns, and the exp'd pair is ADDED into a
            # running [128, 1024] bf16 accumulator (tensor_add at 2x bf16
            # rate); one small reduce per token tile happens in the finals
            npair = len(CHUNKS[i]) // 2
            accs = [
                hpool.tile([128, 2 * CHUNK], BF16, name=f"acc{i}_{ti}")
                for ti in range(nti)
            ]
            for cpair in range(npair):
                wt = wpool.tile([128, kp * CHUNK * 2], FP8, name=f"wt{i}_{cpair}",
                                tag="wt")
                off = kp * CHUNK * 2 * cpair
                nc.gpsimd.dma_start(wt, wT_d[i][:, off:off + kp * CHUNK * 2])
                wt4 = wt.rearrange("p (h g j c) -> p h g j c", h=2, g=ng, j=2)
                for ti in range(nti):
                    ps = psB.tile([128, 2 * CHUNK], F32,
                                  name=f"psB_{i}_{cpair}_{ti}", tag="psB")
                    for h in range(2):
                        for g in range(ng):
                            nc.tensor.matmul(
                                ps[:, h * CHUNK:(h + 1) * CHUNK],
                                lhsT=hidT4[:, g, :, ti * 128:(ti + 1) * 128],
                                rhs=wt4[:, h, g, :, :],
                                start=(g == 0),
                                stop=(g == ng - 1),
                                perf_mode=DR,
                            )
                    scr = spool.tile([128, 2 * CHUNK], BF16,
                                     name=f"scr{i}_{cpair}_{ti}", tag="scr", bufs=5)
                    nc.scalar.activation(scr, ps, Exp, scale=EXP_SCALE)
                    if cpair == 0:
                        nc.vector.tensor_copy(accs[ti], scr)
                    else:
                        nc.vector.tensor_add(accs[ti], accs[ti], scr)

            # ---- Finals: nll = ln(sum exp - n_pad) - logit_t - bias ----
            npad_t = tiny.tile([128, 1], F32, name=f"npad{i}")
            nc.vector.memset(npad_t, float(-NPAD[i]))
            for ti in range(nti):
                S = tiny.tile([128, 1], F32, name=f"S{i}_{ti}")
                nc.vector.reduce_sum(S, accs[ti], axis=X)
                lse = tiny.tile([128, 1], F32, name=f"lse{i}_{ti}")
                nc.scalar.activation(lse, S, Ln, bias=npad_t)
                nllt = tiny.tile([128, 1], F32, name=f"nllt{i}_{ti}")
                nc.scalar.add(nllt, lse, nlts[ti])
                gt = t0 // 128 + ti
                nc.gpsimd.dma_start(out_d[gt:gt + 1, :], nllt)

    nc.finalize()
    return nc


def _get_graph():
    if "nc" not in _GRAPH_CACHE:
        _GRAPH_CACHE["nc"] = _build_graph()
    return _GRAPH_CACHE["nc"]


def _pack_shared(ps, ws):
    """Core-independent packed params (broadcast to every core)."""
    pT_host, wT_host = [], []
    for i in range(3):
        pt = ps[i].T.reshape(KD, 128, PD[i]).transpose(1, 0, 2)  # [128, KD, pd]
        pT_host.append(np.ascontiguousarray(pt).astype(bf16))
        wk = ws[i].T.reshape(KP[i], 128, CSIZE[i]) * np.float32(WSCALE)
        wk = np.concatenate(
            [wk, np.zeros((KP[i], 128, NPAD[i]), np.float32)], axis=2
        )  # pad classes to CPAD with zero weights
        blocks = [
            wk[:, :, off:off + cs].transpose(1, 0, 2).reshape(128, -1)
            for (off, cs) in CHUNKS[i]
        ]
        wT_host.append(np.ascontiguousarray(np.concatenate(blocks, axis=1)).astype(fp8))
    return pT_host, wT_host


def kernel(x, target, p0, w0, b0, p1, w1, b1, p2, w2, b2):
    x = np.asarray(x, dtype=np.float32)
    tgt = np.asarray(target).astype(np.int64)
    ps = [np.asarray(p, np.float32) for p in (p0, p1, p2)]
    ws = [np.asarray(w, np.float32) for w in (w0, w1, w2)]
    bs = [np.asarray(b, np.float32) for b in (b0, b1, b2)]
    N = x.shape[0]

    cid = (tgt >= CUT[1]).astype(np.int32) + (tgt >= CUT[2]).astype(np.int32)
    perm = np.argsort(cid, kind="stable")
    segs = [perm[cid[perm] == i] for i in range(3)]
    core_idx = [np.array_split(segs[i], NCORES) for i in range(3)]
    for i in range(3):
        for j in range(NCORES):
            if len(core_idx[i][j]) > CAP[i]:
                raise RuntimeError(
                    f"cluster {i} capacity exceeded on core {j}: "
                    f"{len(core_idx[i][j])} > {CAP[i]}"
                )

    pT_host, wT_host = _pack_shared(ps, ws)

    in_maps = []
    for j in range(NCORES):
        Xp = np.zeros((TOT, D), np.float32)
        m = {}
        for i in range(3):
            tk = core_idx[i][j]
            l = len(tk)
            Xp[SLOT0[i]:SLOT0[i] + l] = x[tk]
            wg = np.zeros((TILES[i] * 128, PD[i] + 1), np.float32)
            if l:
                local_t = (tgt[tk] - CUT[i]).astype(np.int64)
                wg[:l, :PD[i]] = ws[i][local_t]
                wg[:l, PD[i]] = bs[i][local_t]
            m[f"wg{i}"] = np.ascontiguousarray(
                wg.reshape(TILES[i], 128, PD[i] + 1)
            ).astype(bf16)
            m[f"pT{i}"] = pT_host[i]
            m[f"wT{i}"] = wT_host[i]
        xt = Xp.T.reshape(KD, 128, TOT).transpose(1, 0, 2)  # [128, KD, TOT]
        m["xT"] = np.ascontiguousarray(xt).astype(bf16)
        in_maps.append(m)

    nc = _get_graph()
    res = run_bass_kernel_spmd(nc, in_maps, core_ids=list(range(NCORES)))
    _GRAPH_CACHE["last_results"] = res  # for external profiling harnesses

    nll = np.zeros((N,), np.float32)
    for j in range(NCORES):
        flat = np.asarray(res.results[j]["out"], np.float32).reshape(TOT)
        for i in range(3):
            tk = core_idx[i][j]
            nll[tk] = flat[SLOT0[i]:SLOT0[i] + len(tk)]
    loss = np.float32(nll.sum(dtype=np.float32))
    return loss, nll


# revision 50
# speedup vs baseline: 1.0150x; 1.0150x over previous
# Adaptive softmax (3-cluster) on 8 TRN2 NeuronCores.
#
# Strategy (moe_routing): each token only needs its own cluster's pipeline.
# Host-side we sort tokens by cluster (pure data movement), shard each
# cluster's token segment evenly over the 8 cores, and pad each per-core
# segment to a static capacity so the Bass graph stays shape-static.
# Per core, per cluster c with nt tokens / pd proj dim / C classes:
#   hiddenT[pd, nt] = p_c @ x_shard^T            (PE, bf16)
#   hidden  [nt, pd]                             (PE, bf16; token-major)
#   logits  [nt, C] = hidden @ w_c^T             (PE, fp8 DoubleRow, chunked)
#   sumexp  [nt]    = sum_C exp(logits)          (ScalarE exp -> DVE row-sum)
#   logit_t [nt]    = rowdot(hidden, w_c[tgt]) + b_c[tgt]    (DVE, bf16)
#   nll     [nt]    = ln(sumexp) - logit_t
# The big logits GEMM runs in fp8e4m3 DoubleRow (2 rows/PE cell = 2x MACs).
# fp8 range handling: weights are pre-scaled x64 and hidden x4 (powers of
# two, lossless), and the exp activation rescales with scale=1/256.
# No max-subtraction is needed: |logits| <~ 4 for this problem's scales.
# Classes are padded to uniform 512-wide chunks with zero weights; each
# padded class contributes exactly exp(0)=1, corrected via the Ln pre-bias.
# The target-row weights w_c[tgt] are gathered on host (data movement only).
# No collectives: pure data parallelism; host gathers/unpermutes/sums.

import numpy as np
import ml_dtypes
from contextlib import ExitStack

import concourse.bass as bass
import concourse.bacc as bacc
import concourse.mybir as mybir
import concourse.tile as tile
from concourse.bass_utils import run_bass_kernel_spmd

BF16 = mybir.dt.bfloat16
FP8 = mybir.dt.float8e4
F32 = mybir.dt.float32
bf16 = ml_dtypes.bfloat16
fp8 = ml_dtypes.float8_e4m3

VOCAB = 50257
D = 1024           # input dim
KD = D // 128      # k-tiles over input dim
CUT = [0, 10000, 30000, VOCAB]
PD = [1024, 512, 256]            # per-cluster projection dims
KP = [p // 128 for p in PD]      # k-tiles over proj dim
CSIZE = [CUT[i + 1] - CUT[i] for i in range(3)]
NCORES = 8
CAP = [256, 512, 512]            # per-core token capacity per cluster (padded)
TILES = [c // 128 for c in CAP]
SLOT0 = [0, 256, 768]            # slot offset of each cluster's segment
TOT = sum(CAP)                   # 1280 padded tokens per core
NT = TOT // 128                  # 10 token tiles per core
CHUNK = 512                      # class chunk (one PSUM bank of f32)

HSCALE = 4.0                     # hidden fp8 pre-scale (power of 2)
WSCALE = 64.0                    # weight fp8 pre-scale (power of 2)
EXP_SCALE = 1.0 / (HSCALE * WSCALE)

# class padding to uniform chunks
CPAD = [-(-C // CHUNK) * CHUNK for C in CSIZE]
NPAD = [CPAD[i] - CSIZE[i] for i in range(3)]
CHUNKS = [[(off, CHUNK) for off in range(0, CPAD[i], CHUNK)] for i in range(3)]

# cluster processing order: smallest input first (shrinks the pre-matmul
# head), c0 last (smallest tail)
ORDER = [2, 1, 0]

_GRAPH_CACHE = {}


def _build_graph():
    # Bacc (not plain Bass): its compile() pass splits semaphore waits into
    # event-semaphore carriers, satisfying TRN2's 1-wait-per-instruction limit.
    nc = bacc.Bacc(trn_type="TRN2", target_bir_lowering=False)

    xT_d = nc.dram_tensor("xT", [128, KD, TOT], BF16, kind="ExternalInput")
    pT_d = [
        nc.dram_tensor(f"pT{i}", [128, KD, PD[i]], BF16, kind="ExternalInput")
        for i in range(3)
    ]
    F = [KP[i] * CPAD[i] for i in range(3)]
    wT_d = [
        nc.dram_tensor(f"wT{i}", [128, F[i]], FP8, kind="ExternalInput")
        for i in range(3)
    ]
    # wg carries the gathered target-row weights plus one bias column
    wg_d = [
        nc.dram_tensor(f"wg{i}", [TILES[i], 128, PD[i] + 1], BF16, kind="ExternalInput")
        for i in range(3)
    ]
    out_d = nc.dram_tensor("out", [NT, 128], F32, kind="ExternalOutput")

    Exp = mybir.ActivationFunctionType.Exp
    Ln = mybir.ActivationFunctionType.Ln
    X = mybir.AxisListType.X
    DR = mybir.MatmulPerfMode.DoubleRow

    with ExitStack() as ctx:
        tc = ctx.enter_context(tile.TileContext(nc))
        const = ctx.enter_context(tc.tile_pool(name="const", bufs=1))
        wpool = ctx.enter_context(tc.tile_pool(name="wpool", bufs=8))
        hpool = ctx.enter_context(tc.tile_pool(name="hpool", bufs=1))
        spool = ctx.enter_context(tc.tile_pool(name="spool", bufs=2))
        tiny = ctx.enter_context(tc.tile_pool(name="tiny", bufs=1))
        psA = ctx.enter_context(tc.tile_pool(name="psA", bufs=2, space="PSUM"))
        psB = ctx.enter_context(tc.tile_pool(name="psB", bufs=3, space="PSUM"))

        # input DMAs for all clusters up front (processing order first) so a
        # cluster's pt/xt land before the previous cluster's weight stream
        # monopolizes the DMA lanes
        pts, xts = {}, {}
        for i in ORDER:
            pt = const.tile([128, KD * PD[i]], BF16, name=f"pt{i}")
            nc.gpsimd.dma_start(pt, pT_d[i][:, :, :])
            xt = const.tile([128, KD * CAP[i]], BF16, name=f"xt{i}")
            nc.gpsimd.dma_start(xt, xT_d[:, :, SLOT0[i]:SLOT0[i] + CAP[i]])
            pts[i], xts[i] = pt, xt

        for i in ORDER:
            ntok, t0, kp, pd, nti = CAP[i], SLOT0[i], KP[i], PD[i], TILES[i]
            ng = kp // 2  # DoubleRow contraction groups (K=256 each)

            pt3 = pts[i].rearrange("p (k m) -> p k m", k=KD)
            xt3 = xts[i].rearrange("p (k t) -> p k t", k=KD)

            # ---- Stage A1: hiddenT [pd, ntok] as fp8 (x4), pd-major ----
            hidT = hpool.tile([128, kp * ntok], FP8, name=f"hidT{i}")
            hidT3 = hidT.rearrange("p (k t) -> p k t", k=kp)
            hidT4 = hidT.rearrange("p (g j t) -> p g j t", g=ng, j=2)
            for mp in range(kp):
                ps = psA.tile([128, ntok], F32, name=f"psA1_{i}_{mp}", tag="psA")
                for k in range(KD):
                    nc.tensor.matmul(
                        ps,
                        lhsT=pt3[:, k, mp * 128:(mp + 1) * 128],
                        rhs=xt3[:, k, :],
                        start=(k == 0),
                        stop=(k == KD - 1),
                    )
                # f32 -> fp8 with x4 pre-scale, on ScalarE
                nc.scalar.mul(hidT3[:, mp, :], ps, HSCALE)

            # ---- Stage A2: token-major hidden (bf16) + target-logit dot ----
            nlts = []
            for ti in range(nti):
                ht = hpool.tile([128, pd + 1], BF16, name=f"ht{i}_{ti}")
                nc.vector.memset(ht[:, pd:pd + 1], 1.0)
                for c0 in range(0, pd, 512):
                    cw = min(512, pd - c0)
                    ps = psA.tile([128, cw], F32, name=f"psA2_{i}_{ti}_{c0}", tag="psA")
                    for k in range(KD):
                        nc.tensor.matmul(
                            ps,
                            lhsT=xt3[:, k, ti * 128:(ti + 1) * 128],
                            rhs=pt3[:, k, c0:c0 + cw],
                            start=(k == 0),
                            stop=(k == KD - 1),
                        )
                    nc.vector.tensor_copy(ht[:, c0:c0 + cw], ps)
                wgt = const.tile([128, pd + 1], BF16, name=f"wgt{i}_{ti}")
                nc.gpsimd.dma_start(wgt, wg_d[i][ti, :, :])
                # absorb the DMA wait on a TensorCopy so the TensorTensor
                # below needs at most one sem wait
                dm = tiny.tile([128, 1], BF16, name=f"dm{i}_{ti}")
                nc.vector.tensor_copy(dm, wgt[:, 0:1])
                prod = spool.tile([128, pd + 1], F32, name=f"prod{i}_{ti}", tag="prod")
                nlt = tiny.tile([128, 1], F32, name=f"nlt{i}_{ti}")
                nc.vector.tensor_mul(prod, ht, wgt)
                nc.vector.reduce_sum(nlt, prod, axis=X, negate=True)
                nlts.append(nlt)

            # ---- Stage B: fp8 DoubleRow logits + exp + running bf16 sum ----
            # chunks processed in pairs sharing one 2-bank PSUM tile; the exp
            # runs once per 1024 columns, and the exp'd pair is ADDED into a
            # running [128, 1024] bf16 accumulator (tensor_add at 2x bf16
            # rate); one small reduce per token tile happens in the finals
            npair = len(CHUNKS[i]) // 2
            accs = [
                hpool.tile([128, 2 * CHUNK], BF16, name=f"acc{i}_{ti}")
                for ti in range(nti)
            ]
            for cpair in range(npair):
                wt = wpool.tile([128, kp * CHUNK * 2], FP8, name=f"wt{i}_{cpair}",
                                tag="wt")
                off = kp * CHUNK * 2 * cpair
                nc.gpsimd.dma_start(wt, wT_d[i][:, off:off + kp * CHUNK * 2])
                wt4 = wt.rearrange("p (h g j c) -> p h g j c", h=2, g=ng, j=2)
                for ti in range(nti):
                    ps = psB.tile([128, 2 * CHUNK], F32,
                                  name=f"psB_{i}_{cpair}_{ti}", tag="psB")
                    for h in range(2):
                        for g in range(ng):
                            nc.tensor.matmul(
                                ps[:, h * CHUNK:(h + 1) * CHUNK],
                                lhsT=hidT4[:, g, :, ti * 128:(ti + 1) * 128],
                                rhs=wt4[:, h, g, :, :],
                                start=(g == 0),
                                stop=(g == ng - 1),
                                perf_mode=DR,
                            )
                    scr = spool.tile([128, 2 * CHUNK], BF16,
                                     name=f"scr{i}_{cpair}_{ti}", tag="scr", bufs=3)
                    nc.scalar.activation(scr, ps, Exp, scale=EXP_SCALE)
                    if cpair == 0:
                        nc.vector.tensor_copy(accs[ti], scr)
                    else:
                        nc.vector.tensor_add(accs[ti], accs[ti], scr)

            # ---- Finals: nll = ln(sum exp - n_pad) - logit_t - bias ----
            npad_t = tiny.tile([128, 1], F32, name=f"npad{i}")
            nc.vector.memset(npad_t, float(-NPAD[i]))
            for ti in range(nti):
                S = tiny.tile([128, 1], F32, name=f"S{i}_{ti}")
                nc.vector.reduce_sum(S, accs[ti], axis=X)
                lse = tiny.tile([128, 1], F32, name=f"lse{i}_{ti}")
                nc.scalar.activation(lse, S, Ln, bias=npad_t)
                nllt = tiny.tile([128, 1], F32, name=f"nllt{i}_{ti}")
                nc.scalar.add(nllt, lse, nlts[ti])
                gt = t0 // 128 + ti
                nc.gpsimd.dma_start(out_d[gt:gt + 1, :], nllt)

    nc.finalize()
    return nc


def _get_graph():
    if "nc" not in _GRAPH_CACHE:
        _GRAPH_CACHE["nc"] = _build_graph()
    return _GRAPH_CACHE["nc"]


def _pack_shared(ps, ws):
    """Core-independent packed params (broadcast to every core)."""
    pT_host, wT_host = [], []
    for i in range(3):
        pt = ps[i].T.reshape(KD, 128, PD[i]).transpose(1, 0, 2)  # [128, KD, pd]
        pT_host.append(np.ascontiguousarray(pt).astype(bf16))
        wk = ws[i].T.reshape(KP[i], 128, CSIZE[i]) * np.float32(WSCALE)
        wk = np.concatenate(
            [wk, np.zeros((KP[i], 128, NPAD[i]), np.float32)], axis=2
        )  # pad classes to CPAD with zero weights
        blocks = [
            wk[:, :, off:off + cs].transpose(1, 0, 2).reshape(128, -1)
            for (off, cs) in CHUNKS[i]
        ]
        wT_host.append(np.ascontiguousarray(np.concatenate(blocks, axis=1)).astype(fp8))
    return pT_host, wT_host


def kernel(x, target, p0, w0, b0, p1, w1, b1, p2, w2, b2):
    x = np.asarray(x, dtype=np.float32)
    tgt = np.asarray(target).astype(np.int64)
    ps = [np.asarray(p, np.float32) for p in (p0, p1, p2)]
    ws = [np.asarray(w, np.float32) for w in (w0, w1, w2)]
    bs = [np.asarray(b, np.float32) for b in (b0, b1, b2)]
    N = x.shape[0]

    cid = (tgt >= CUT[1]).astype(np.int32) + (tgt >= CUT[2]).astype(np.int32)
    perm = np.argsort(cid, kind="stable")
    segs = [perm[cid[perm] == i] for i in range(3)]
    core_idx = [np.array_split(segs[i], NCORES) for i in range(3)]
    for i in range(3):
        for j in range(NCORES):
            if len(core_idx[i][j]) > CAP[i]:
                raise RuntimeError(
                    f"cluster {i} capacity exceeded on core {j}: "
                    f"{len(core_idx[i][j])} > {CAP[i]}"
                )

    pT_host, wT_host = _pack_shared(ps, ws)

    in_maps = []
    for j in range(NCORES):
        Xp = np.zeros((TOT, D), np.float32)
        m = {}
        for i in range(3):
            tk = core_idx[i][j]
            l = len(tk)
            Xp[SLOT0[i]:SLOT0[i] + l] = x[tk]
            wg = np.zeros((TILES[i] * 128, PD[i] + 1), np.float32)
            if l:
                local_t = (tgt[tk] - CUT[i]).astype(np.int64)
                wg[:l, :PD[i]] = ws[i][local_t]
                wg[:l, PD[i]] = bs[i][local_t]
            m[f"wg{i}"] = np.ascontiguousarray(
                wg.reshape(TILES[i], 128, PD[i] + 1)
            ).astype(bf16)
            m[f"pT{i}"] = pT_host[i]
            m[f"wT{i}"] = wT_host[i]
        xt = Xp.T.reshape(KD, 128, TOT).transpose(1, 0, 2)  # [128, KD, TOT]
        m["xT"] = np.ascontiguousarray(xt).astype(bf16)
        in_maps.append(m)

    nc = _get_graph()
    res = run_bass_kernel_spmd(nc, in_maps, core_ids=list(range(NCORES)))
    _GRAPH_CACHE["last_results"] = res  # for external profiling harnesses

    nll = np.zeros((N,), np.float32)
    for j in range(NCORES):
        flat = np.asarray(res.results[j]["out"], np.float32).reshape(TOT)
        for i in range(3):
            tk = core_idx[i][j]
            nll[tk] = flat[SLOT0[i]:SLOT0[i] + len(tk)]
    loss = np.float32(nll.sum(dtype=np.float32))
    return loss, nll


# revision 51
# speedup vs baseline: 1.0557x; 1.0400x over previous
# Adaptive softmax (3-cluster) on 8 TRN2 NeuronCores.
#
# Strategy (moe_routing): each token only needs its own cluster's pipeline.
# Host-side we sort tokens by cluster (pure data movement), shard each
# cluster's token segment evenly over the 8 cores, and pad each per-core
# segment to a static capacity so the Bass graph stays shape-static.
# Per core, per cluster c with nt tokens / pd proj dim / C classes:
#   hiddenT[pd, nt] = p_c @ x_shard^T            (PE, bf16)
#   hidden  [nt, pd]                             (PE, bf16; token-major)
#   logits  [nt, C] = hidden @ w_c^T             (PE, fp8 DoubleRow, chunked)
#   sumexp  [nt]    = sum_C exp(logits)          (ScalarE exp -> DVE 2x adds)
#   logit_t [nt]    = rowdot(hidden, w_c[tgt]) + b_c[tgt]    (DVE, bf16)
#   nll     [nt]    = ln(sumexp) - logit_t
# To cut token padding, c1/c2 are capped at 3 tiles (384) per core and the
# overflow tokens go to one generic "X tile" per core: a pd=512 /
# 20480-class pipeline whose projection/classifier weights are per-core
# inputs -- either cluster 1's natively or cluster 2's zero-padded from
# pd 256 to 512 (numerically exact), so the SPMD graph stays identical.
# The big logits GEMM runs in fp8e4m3 DoubleRow (2 rows/PE cell = 2x MACs).
# fp8 range handling: weights are pre-scaled x64 and hidden x4 (powers of
# two, lossless), and the exp activation rescales with scale=1/256.
# No max-subtraction is needed: |logits| <~ 4 for this problem's scales.
# Classes are padded to uniform 512-wide chunks with zero weights; each
# padded class contributes exactly exp(0)=1, corrected via the Ln pre-bias.
# The target-row weights w_c[tgt] are gathered on host (data movement only).
# No collectives: pure data parallelism; host gathers/unpermutes/sums.

import numpy as np
import ml_dtypes
from contextlib import ExitStack

import concourse.bass as bass
import concourse.bacc as bacc
import concourse.mybir as mybir
import concourse.tile as tile
from concourse.bass_utils import run_bass_kernel_spmd

BF16 = mybir.dt.bfloat16
FP8 = mybir.dt.float8e4
F32 = mybir.dt.float32
bf16 = ml_dtypes.bfloat16
fp8 = ml_dtypes.float8_e4m3

VOCAB = 50257
D = 1024           # input dim
KD = D // 128      # k-tiles over input dim
CUT = [0, 10000, 30000, VOCAB]
PD = [1024, 512, 256]            # per-cluster projection dims
KP = [p // 128 for p in PD]      # k-tiles over proj dim
CSIZE = [CUT[i + 1] - CUT[i] for i in range(3)]
NCORES = 8
CAP = [256, 384, 384]            # per-core token capacity per cluster
TILES = [c // 128 for c in CAP]
SLOT0 = [0, 256, 640]            # slot offset of each cluster's segment
XCAP = 128                       # one generic overflow tile per core
XSLOT = 1024
XPD = 512                        # X pipeline covers pd<=512 (c1 or c2-padded)
XKP = XPD // 128
TOT = sum(CAP) + XCAP            # 1152 padded tokens per core
NT = TOT // 128                  # 9 token tiles per core
CHUNK = 512                      # class chunk (one PSUM bank of f32)

HSCALE = 4.0                     # hidden fp8 pre-scale (power of 2)
WSCALE = 64.0                    # weight fp8 pre-scale (power of 2)
EXP_SCALE = 1.0 / (HSCALE * WSCALE)

# class padding to uniform chunks
CPAD = [-(-C // CHUNK) * CHUNK for C in CSIZE]
NPAD = [CPAD[i] - CSIZE[i] for i in range(3)]
CHUNKS = [[(off, CHUNK) for off in range(0, CPAD[i], CHUNK)] for i in range(3)]
assert CPAD[1] == CPAD[2] == 20480  # X pipeline class width
XCPAD = CPAD[1]
XF = XKP * XCPAD

# cluster processing order: smallest input first (shrinks the pre-matmul
# head); X between c1 and c0; c0 last (smallest tail)
ORDER = [2, 1, 0]

_GRAPH_CACHE = {}


def _build_graph():
    # Bacc (not plain Bass): its compile() pass splits semaphore waits into
    # event-semaphore carriers, satisfying TRN2's 1-wait-per-instruction limit.
    nc = bacc.Bacc(trn_type="TRN2", target_bir_lowering=False)

    xT_d = nc.dram_tensor("xT", [128, KD, TOT], BF16, kind="ExternalInput")
    pT_d = [
        nc.dram_tensor(f"pT{i}", [128, KD, PD[i]], BF16, kind="ExternalInput")
        for i in range(3)
    ]
    F = [KP[i] * CPAD[i] for i in range(3)]
    wT_d = [
        nc.dram_tensor(f"wT{i}", [128, F[i]], FP8, kind="ExternalInput")
        for i in range(3)
    ]
    # wg carries the gathered target-row weights plus one bias column
    wg_d = [
        nc.dram_tensor(f"wg{i}", [TILES[i], 128, PD[i] + 1], BF16, kind="ExternalInput")
        for i in range(3)
    ]
    # X-tile (overflow) pipeline inputs: per-core cluster choice baked by host
    pX_d = nc.dram_tensor("pX", [128, KD, XPD], BF16, kind="ExternalInput")
    wX_d = nc.dram_tensor("wX", [128, XF], FP8, kind="ExternalInput")
    wgX_d = nc.dram_tensor("wgX", [1, 128, XPD + 1], BF16, kind="ExternalInput")
    npX_d = nc.dram_tensor("npX", [128, 1], F32, kind="ExternalInput")
    out_d = nc.dram_tensor("out", [NT, 128], F32, kind="ExternalOutput")

    Exp = mybir.ActivationFunctionType.Exp
    Ln = mybir.ActivationFunctionType.Ln
    X = mybir.AxisListType.X
    DR = mybir.MatmulPerfMode.DoubleRow

    # (tag, ntok, t0, kp, pd, nti, nch, pT, wT, wg, npad_const)
    jobs_order = []
    for i in ORDER:
        jobs_order.append(dict(
            tag=f"{i}", ntok=CAP[i], t0=SLOT0[i], kp=KP[i], pd=PD[i],
            nti=TILES[i], nch=len(CHUNKS[i]), pT=pT_d[i], wT=wT_d[i],
            wg=wg_d[i], npad=float(-NPAD[i]),
        ))
    xjob = dict(
        tag="x", ntok=XCAP, t0=XSLOT, kp=XKP, pd=XPD, nti=1,
        nch=len(CHUNKS[1]), pT=pX_d, wT=wX_d, wg=wgX_d, npad=None,
    )
    jobs_order.insert(2, xjob)  # c2, c1, X, c0

    with ExitStack() as ctx:
        tc = ctx.enter_context(tile.TileContext(nc))
        const = ctx.enter_context(tc.tile_pool(name="const", bufs=1))
        wpool = ctx.enter_context(tc.tile_pool(name="wpool", bufs=8))
        hpool = ctx.enter_context(tc.tile_pool(name="hpool", bufs=1))
        spool = ctx.enter_context(tc.tile_pool(name="spool", bufs=2))
        tiny = ctx.enter_context(tc.tile_pool(name="tiny", bufs=1))
        psA = ctx.enter_context(tc.tile_pool(name="psA", bufs=2, space="PSUM"))
        psB = ctx.enter_context(tc.tile_pool(name="psB", bufs=3, space="PSUM"))

        # input DMAs for all jobs up front (processing order first) so a
        # job's pt/xt land before the previous job's weight stream
        # monopolizes the DMA lanes
        for jb in jobs_order:
            pt = const.tile([128, KD * jb["pd"]], BF16, name=f"pt{jb['tag']}")
            nc.gpsimd.dma_start(pt, jb["pT"][:, :, :])
            xt = const.tile([128, KD * jb["ntok"]], BF16, name=f"xt{jb['tag']}")
            nc.gpsimd.dma_start(xt, xT_d[:, :, jb["t0"]:jb["t0"] + jb["ntok"]])
            jb["pt3"] = pt.rearrange("p (k m) -> p k m", k=KD)
            jb["xt3"] = xt.rearrange("p (k t) -> p k t", k=KD)

        for jb in jobs_order:
            tag, ntok, t0 = jb["tag"], jb["ntok"], jb["t0"]
            kp, pd, nti, nch = jb["kp"], jb["pd"], jb["nti"], jb["nch"]
            ng = kp // 2  # DoubleRow contraction groups (K=256 each)
            pt3, xt3 = jb["pt3"], jb["xt3"]

            # ---- Stage A1: hiddenT [pd, ntok] as fp8 (x4), pd-major ----
            hidT = hpool.tile([128, kp * ntok], FP8, name=f"hidT{tag}")
            hidT3 = hidT.rearrange("p (k t) -> p k t", k=kp)
            hidT4 = hidT.rearrange("p (g j t) -> p g j t", g=ng, j=2)
            for mp in range(kp):
                ps = psA.tile([128, ntok], F32, name=f"psA1_{tag}_{mp}", tag="psA")
                for k in range(KD):
                    nc.tensor.matmul(
                        ps,
                        lhsT=pt3[:, k, mp * 128:(mp + 1) * 128],
                        rhs=xt3[:, k, :],
                        start=(k == 0),
                        stop=(k == KD - 1),
                    )
                # f32 -> fp8 with x4 pre-scale, on ScalarE
                nc.scalar.mul(hidT3[:, mp, :], ps, HSCALE)

            # ---- Stage A2: token-major hidden (bf16) + target-logit dot ----
            nlts = []
            for ti in range(nti):
                ht = hpool.tile([128, pd + 1], BF16, name=f"ht{tag}_{ti}")
                nc.vector.memset(ht[:, pd:pd + 1], 1.0)
                for c0 in range(0, pd, 512):
                    cw = min(512, pd - c0)
                    ps = psA.tile([128, cw], F32, name=f"psA2_{tag}_{ti}_{c0}",
                                  tag="psA")
                    for k in range(KD):
                        nc.tensor.matmul(
                            ps,
                            lhsT=xt3[:, k, ti * 128:(ti + 1) * 128],
                            rhs=pt3[:, k, c0:c0 + cw],
                            start=(k == 0),
                            stop=(k == KD - 1),
                        )
                    nc.vector.tensor_copy(ht[:, c0:c0 + cw], ps)
                wgt = const.tile([128, pd + 1], BF16, name=f"wgt{tag}_{ti}")
                nc.gpsimd.dma_start(wgt, jb["wg"][ti, :, :])
                # absorb the DMA wait on a TensorCopy so the TensorTensor
                # below needs at most one sem wait
                dm = tiny.tile([128, 1], BF16, name=f"dm{tag}_{ti}")
                nc.vector.tensor_copy(dm, wgt[:, 0:1])
                prod = spool.tile([128, pd + 1], F32, name=f"prod{tag}_{ti}",
                                  tag="prod")
                nlt = tiny.tile([128, 1], F32, name=f"nlt{tag}_{ti}")
                nc.vector.tensor_mul(prod, ht, wgt)
                nc.vector.reduce_sum(nlt, prod, axis=X, negate=True)
                nlts.append(nlt)

            # ---- Stage B: fp8 DoubleRow logits + exp + running bf16 sum ----
            # chunks processed in pairs sharing one 2-bank PSUM tile; the exp
            # runs once per 1024 columns, and the exp'd pair is ADDED into a
            # running [128, 1024] bf16 accumulator (tensor_add at 2x bf16
            # rate); one small reduce per token tile happens in the finals
            npair = nch // 2
            accs = [
                hpool.tile([128, 2 * CHUNK], BF16, name=f"acc{tag}_{ti}")
                for ti in range(nti)
            ]
            for cpair in range(npair):
                wt = wpool.tile([128, kp * CHUNK * 2], FP8, name=f"wt{tag}_{cpair}",
                                tag="wt")
                off = kp * CHUNK * 2 * cpair
                nc.gpsimd.dma_start(wt, jb["wT"][:, off:off + kp * CHUNK * 2])
                wt4 = wt.rearrange("p (h g j c) -> p h g j c", h=2, g=ng, j=2)
                for ti in range(nti):
                    ps = psB.tile([128, 2 * CHUNK], F32,
                                  name=f"psB_{tag}_{cpair}_{ti}", tag="psB")
                    for h in range(2):
                        for g in range(ng):
                            nc.tensor.matmul(
                                ps[:, h * CHUNK:(h + 1) * CHUNK],
                                lhsT=hidT4[:, g, :, ti * 128:(ti + 1) * 128],
                                rhs=wt4[:, h, g, :, :],
                                start=(g == 0),
                                stop=(g == ng - 1),
                                perf_mode=DR,
                            )
                    scr = spool.tile([128, 2 * CHUNK], BF16,
                                     name=f"scr{tag}_{cpair}_{ti}", tag="scr",
                                     bufs=3)
                    nc.scalar.activation(scr, ps, Exp, scale=EXP_SCALE)
                    if cpair == 0:
                        nc.vector.tensor_copy(accs[ti], scr)
                    else:
                        nc.vector.tensor_add(accs[ti], accs[ti], scr)

            # ---- Finals: nll = ln(sum exp - n_pad) - logit_t - bias ----
            npad_t = const.tile([128, 1], F32, name=f"npad{tag}")
            if jb["npad"] is None:
                nc.gpsimd.dma_start(npad_t, npX_d[:, :])
            else:
                nc.vector.memset(npad_t, jb["npad"])
            for ti in range(nti):
                S = tiny.tile([128, 1], F32, name=f"S{tag}_{ti}")
                nc.vector.reduce_sum(S, accs[ti], axis=X)
                lse = tiny.tile([128, 1], F32, name=f"lse{tag}_{ti}")
                nc.scalar.activation(lse, S, Ln, bias=npad_t)
                nllt = tiny.tile([128, 1], F32, name=f"nllt{tag}_{ti}")
                nc.scalar.add(nllt, lse, nlts[ti])
                gt = t0 // 128 + ti
                nc.gpsimd.dma_start(out_d[gt:gt + 1, :], nllt)

    nc.finalize()
    return nc


def _get_graph():
    if "nc" not in _GRAPH_CACHE:
        _GRAPH_CACHE["nc"] = _build_graph()
    return _GRAPH_CACHE["nc"]


def _pack_pT(p):
    """[pd, D] f32 -> [128, KD, pd] bf16 (d-major partition tiles)."""
    pt = p.T.reshape(KD, 128, p.shape[0]).transpose(1, 0, 2)
    return np.ascontiguousarray(pt).astype(bf16)


def _pack_wT(w, kp, cpad):
    """[C, pd] f32 -> [128, kp*cpad] fp8 (x64), chunk-contiguous blocks."""
    pd, C = w.shape[1], w.shape[0]
    wk = w.T.reshape(kp, 128, C) * np.float32(WSCALE)
    wk = np.concatenate([wk, np.zeros((kp, 128, cpad - C), np.float32)], axis=2)
    blocks = [
        wk[:, :, off:off + CHUNK].transpose(1, 0, 2).reshape(128, -1)
        for off in range(0, cpad, CHUNK)
    ]
    return np.ascontiguousarray(np.concatenate(blocks, axis=1)).astype(fp8)


def _pack_shared(ps, ws):
    """Core-independent packed params (broadcast to every core)."""
    pT_host = [_pack_pT(ps[i]) for i in range(3)]
    wT_host = [_pack_wT(ws[i], KP[i], CPAD[i]) for i in range(3)]
    # X-pipeline variants: cluster 1 native (pd 512); cluster 2 zero-padded
    # from pd 256 to 512 (exact)
    p2pad = np.concatenate([ps[2], np.zeros((XPD - PD[2], D), np.float32)], axis=0)
    w2pad = np.concatenate(
        [ws[2], np.zeros((CSIZE[2], XPD - PD[2]), np.float32)], axis=1
    )
    pX_var = {1: _pack_pT(ps[1]), 2: _pack_pT(p2pad)}
    wX_var = {1: wT_host[1], 2: _pack_wT(w2pad, XKP, XCPAD)}
    return pT_host, wT_host, pX_var, wX_var


def kernel(x, target, p0, w0, b0, p1, w1, b1, p2, w2, b2):
    x = np.asarray(x, dtype=np.float32)
    tgt = np.asarray(target).astype(np.int64)
    ps = [np.asarray(p, np.float32) for p in (p0, p1, p2)]
    ws = [np.asarray(w, np.float32) for w in (w0, w1, w2)]
    bs = [np.asarray(b, np.float32) for b in (b0, b1, b2)]
    N = x.shape[0]

    cid = (tgt >= CUT[1]).astype(np.int32) + (tgt >= CUT[2]).astype(np.int32)
    perm = np.argsort(cid, kind="stable")
    segs = [perm[cid[perm] == i] for i in range(3)]

    # c0: even split over cores (capacity 256 each). c1/c2: exactly 384 per
    # core, remainder routed to the per-core X tiles.
    core_idx = [[None] * NCORES for _ in range(3)]
    core_idx[0] = np.array_split(segs[0], NCORES)
    for j in range(NCORES):
        if len(core_idx[0][j]) > CAP[0]:
            raise RuntimeError("cluster 0 capacity exceeded")
    xassign = []  # per X tile: (cluster, token ids)
    for i in (1, 2):
        seg = segs[i]
        main = CAP[i] * NCORES
        if len(seg) < main:
            # underflow: pad-split evenly, no X overflow for this cluster
            core_idx[i] = np.array_split(seg, NCORES)
        else:
            for j in range(NCORES):
                core_idx[i][j] = seg[j * CAP[i]:(j + 1) * CAP[i]]
            rem = seg[main:]
            for off in range(0, len(rem), XCAP):
                xassign.append((i, rem[off:off + XCAP]))
    if len(xassign) > NCORES:
        raise RuntimeError(f"X-tile capacity exceeded: {len(xassign)} > {NCORES}")
    while len(xassign) < NCORES:
        xassign.append((1, np.array([], dtype=np.int64)))

    pT_host, wT_host, pX_var, wX_var = _pack_shared(ps, ws)
    npX_var = {
        xc: np.full((128, 1), float(-NPAD[xc]), np.float32) for xc in (1, 2)
    }

    in_maps = []
    for j in range(NCORES):
        Xp = np.zeros((TOT, D), np.float32)
        m = {}
        for i in range(3):
            tk = core_idx[i][j]
            l = len(tk)
            Xp[SLOT0[i]:SLOT0[i] + l] = x[tk]
            wg = np.zeros((TILES[i] * 128, PD[i] + 1), np.float32)
            if l:
                local_t = (tgt[tk] - CUT[i]).astype(np.int64)
                wg[:l, :PD[i]] = ws[i][local_t]
                wg[:l, PD[i]] = bs[i][local_t]
            m[f"wg{i}"] = np.ascontiguousarray(
                wg.reshape(TILES[i], 128, PD[i] + 1)
            ).astype(bf16)
            m[f"pT{i}"] = pT_host[i]
            m[f"wT{i}"] = wT_host[i]
        xc, xtk = xassign[j]
        lx = len(xtk)
        if lx:
            Xp[XSLOT:XSLOT + lx] = x[xtk]
        wgx = np.zeros((128, XPD + 1), np.float32)
        if lx:
            local_t = (tgt[xtk] - CUT[xc]).astype(np.int64)
            wgx[:lx, :PD[xc]] = ws[xc][local_t]
            wgx[:lx, XPD] = bs[xc][local_t]
        m["wgX"] = np.ascontiguousarray(wgx.reshape(1, 128, XPD + 1)).astype(bf16)
        m["pX"] = pX_var[xc]
        m["wX"] = wX_var[xc]
        m["npX"] = npX_var[xc]
        xt = Xp.T.reshape(KD, 128, TOT).transpose(1, 0, 2)  # [128, KD, TOT]
        m["xT"] = np.ascontiguousarray(xt).astype(bf16)
        in_maps.append(m)

    nc = _get_graph()
    res = run_bass_kernel_spmd(nc, in_maps, core_ids=list(range(NCORES)))
    _GRAPH_CACHE["last_results"] = res  # for external profiling harnesses

    nll = np.zeros((N,), np.float32)
    for j in range(NCORES):
        flat = np.asarray(res.results[j]["out"], np.float32).reshape(TOT)
        for i in range(3):
            tk = core_idx[i][j]
            nll[tk] = flat[SLOT0[i]:SLOT0[i] + len(tk)]
        xc, xtk = xassign[j]
        if len(xtk):
            nll[xtk] = flat[XSLOT:XSLOT + len(xtk)]
    loss = np.float32(nll.sum(dtype=np.float32))
    return loss, nll


# revision 53
# speedup vs baseline: 1.0596x; 1.0037x over previous
# Adaptive softmax (3-cluster) on 8 TRN2 NeuronCores.
#
# Strategy (moe_routing): each token only needs its own cluster's pipeline.
# Host-side we sort tokens by cluster (pure data movement), shard each
# cluster's token segment evenly over the 8 cores, and pad each per-core
# segment to a static capacity so the Bass graph stays shape-static.
# Per core, per cluster c with nt tokens / pd proj dim / C classes:
#   hiddenT[pd, nt] = p_c @ x_shard^T            (PE, bf16)
#   hidden  [nt, pd]                             (PE, bf16; token-major)
#   logits  [nt, C] = hidden @ w_c^T             (PE, fp8 DoubleRow, chunked)
#   sumexp  [nt]    = sum_C exp(logits)          (ScalarE exp -> DVE 2x adds)
#   logit_t [nt]    = rowdot(hidden, w_c[tgt]) + b_c[tgt]    (DVE, bf16)
#   nll     [nt]    = ln(sumexp) - logit_t
# To cut token padding, c1/c2 are capped at 3 tiles (384) per core and the
# overflow tokens go to one generic "X tile" per core: a pd=512 /
# 20480-class pipeline whose projection/classifier weights are per-core
# inputs -- either cluster 1's natively or cluster 2's zero-padded from
# pd 256 to 512 (numerically exact), so the SPMD graph stays identical.
# The big logits GEMM runs in fp8e4m3 DoubleRow (2 rows/PE cell = 2x MACs).
# fp8 range handling: weights are pre-scaled x64 and hidden x4 (powers of
# two, lossless), and the exp activation rescales with scale=1/256.
# No max-subtraction is needed: |logits| <~ 4 for this problem's scales.
# Classes are padded to uniform 512-wide chunks with zero weights; each
# padded class contributes exactly exp(0)=1, corrected via the Ln pre-bias.
# The target-row weights w_c[tgt] are gathered on host (data movement only).
# No collectives: pure data parallelism; host gathers/unpermutes/sums.

import numpy as np
import ml_dtypes
from contextlib import ExitStack

import concourse.bass as bass
import concourse.bacc as bacc
import concourse.mybir as mybir
import concourse.tile as tile
from concourse.bass_utils import run_bass_kernel_spmd

BF16 = mybir.dt.bfloat16
FP8 = mybir.dt.float8e4
F32 = mybir.dt.float32
bf16 = ml_dtypes.bfloat16
fp8 = ml_dtypes.float8_e4m3

VOCAB = 50257
D = 1024           # input dim
KD = D // 128      # k-tiles over input dim
CUT = [0, 10000, 30000, VOCAB]
PD = [1024, 512, 256]            # per-cluster projection dims
KP = [p // 128 for p in PD]      # k-tiles over proj dim
CSIZE = [CUT[i + 1] - CUT[i] for i in range(3)]
NCORES = 8
CAP = [256, 384, 384]            # per-core token capacity per cluster
TILES = [c // 128 for c in CAP]
SLOT0 = [0, 256, 640]            # slot offset of each cluster's segment
XCAP = 128                       # one generic overflow tile per core
XSLOT = 1024
XPD = 512                        # X pipeline covers pd<=512 (c1 or c2-padded)
XKP = XPD // 128
TOT = sum(CAP) + XCAP            # 1152 padded tokens per core
NT = TOT // 128                  # 9 token tiles per core
CHUNK = 512                      # class chunk (one PSUM bank of f32)

HSCALE = 4.0                     # hidden fp8 pre-scale (power of 2)
WSCALE = 64.0                    # weight fp8 pre-scale (power of 2)
EXP_SCALE = 1.0 / (HSCALE * WSCALE)

# class padding to uniform chunks
CPAD = [-(-C // CHUNK) * CHUNK for C in CSIZE]
NPAD = [CPAD[i] - CSIZE[i] for i in range(3)]
CHUNKS = [[(off, CHUNK) for off in range(0, CPAD[i], CHUNK)] for i in range(3)]
assert CPAD[1] == CPAD[2] == 20480  # X pipeline class width
XCPAD = CPAD[1]
XF = XKP * XCPAD

# cluster processing order: smallest input first (shrinks the pre-matmul
# head); X between c1 and c0; c0 last (smallest tail)
ORDER = [2, 1, 0]

_GRAPH_CACHE = {}


def _build_graph():
    # Bacc (not plain Bass): its compile() pass splits semaphore waits into
    # event-semaphore carriers, satisfying TRN2's 1-wait-per-instruction limit.
    nc = bacc.Bacc(trn_type="TRN2", target_bir_lowering=False)

    xT_d = nc.dram_tensor("xT", [128, KD, TOT], BF16, kind="ExternalInput")
    pT_d = [
        nc.dram_tensor(f"pT{i}", [128, KD, PD[i]], BF16, kind="ExternalInput")
        for i in range(3)
    ]
    F = [KP[i] * CPAD[i] for i in range(3)]
    wT_d = [
        nc.dram_tensor(f"wT{i}", [128, F[i]], FP8, kind="ExternalInput")
        for i in range(3)
    ]
    # wg carries the gathered target-row weights plus one bias column
    wg_d = [
        nc.dram_tensor(f"wg{i}", [TILES[i], 128, PD[i] + 1], BF16, kind="ExternalInput")
        for i in range(3)
    ]
    # X-tile (overflow) pipeline inputs: per-core cluster choice baked by host
    pX_d = nc.dram_tensor("pX", [128, KD, XPD], BF16, kind="ExternalInput")
    wX_d = nc.dram_tensor("wX", [128, XF], FP8, kind="ExternalInput")
    wgX_d = nc.dram_tensor("wgX", [1, 128, XPD + 1], BF16, kind="ExternalInput")
    npX_d = nc.dram_tensor("npX", [128, 1], F32, kind="ExternalInput")
    out_d = nc.dram_tensor("out", [NT, 128], F32, kind="ExternalOutput")

    Exp = mybir.ActivationFunctionType.Exp
    Ln = mybir.ActivationFunctionType.Ln
    X = mybir.AxisListType.X
    DR = mybir.MatmulPerfMode.DoubleRow

    # (tag, ntok, t0, kp, pd, nti, nch, pT, wT, wg, npad_const)
    jobs_order = []
    for i in ORDER:
        jobs_order.append(dict(
            tag=f"{i}", ntok=CAP[i], t0=SLOT0[i], kp=KP[i], pd=PD[i],
            nti=TILES[i], nch=len(CHUNKS[i]), pT=pT_d[i], wT=wT_d[i],
            wg=wg_d[i], npad=float(-NPAD[i]),
        ))
    xjob = dict(
        tag="x", ntok=XCAP, t0=XSLOT, kp=XKP, pd=XPD, nti=1,
        nch=len(CHUNKS[1]), pT=pX_d, wT=wX_d, wg=wgX_d, npad=None,
    )
    jobs_order.append(xjob)  # c2, c1, c0, X (single-tile X job = short tail)

    with ExitStack() as ctx:
        tc = ctx.enter_context(tile.TileContext(nc))
        const = ctx.enter_context(tc.tile_pool(name="const", bufs=1))
        wpool = ctx.enter_context(tc.tile_pool(name="wpool", bufs=8))
        hpool = ctx.enter_context(tc.tile_pool(name="hpool", bufs=1))
        spool = ctx.enter_context(tc.tile_pool(name="spool", bufs=2))
        tiny = ctx.enter_context(tc.tile_pool(name="tiny", bufs=1))
        psA = ctx.enter_context(tc.tile_pool(name="psA", bufs=2, space="PSUM"))
        psB = ctx.enter_context(tc.tile_pool(name="psB", bufs=3, space="PSUM"))

        # input DMAs for all jobs up front (processing order first) so a
        # job's pt/xt land before the previous job's weight stream
        # monopolizes the DMA lanes
        for jb in jobs_order:
            pt = const.tile([128, KD * jb["pd"]], BF16, name=f"pt{jb['tag']}")
            nc.gpsimd.dma_start(pt, jb["pT"][:, :, :])
            xt = const.tile([128, KD * jb["ntok"]], BF16, name=f"xt{jb['tag']}")
            nc.gpsimd.dma_start(xt, xT_d[:, :, jb["t0"]:jb["t0"] + jb["ntok"]])
            jb["pt3"] = pt.rearrange("p (k m) -> p k m", k=KD)
            jb["xt3"] = xt.rearrange("p (k t) -> p k t", k=KD)

        for jb in jobs_order:
            tag, ntok, t0 = jb["tag"], jb["ntok"], jb["t0"]
            kp, pd, nti, nch = jb["kp"], jb["pd"], jb["nti"], jb["nch"]
            ng = kp // 2  # DoubleRow contraction groups (K=256 each)
            pt3, xt3 = jb["pt3"], jb["xt3"]

            # ---- Stage A1: hiddenT [pd, ntok] as fp8 (x4), pd-major ----
            hidT = hpool.tile([128, kp * ntok], FP8, name=f"hidT{tag}")
            hidT3 = hidT.rearrange("p (k t) -> p k t", k=kp)
            hidT4 = hidT.rearrange("p (g j t) -> p g j t", g=ng, j=2)
            for mp in range(kp):
                ps = psA.tile([128, ntok], F32, name=f"psA1_{tag}_{mp}", tag="psA")
                for k in range(KD):
                    nc.tensor.matmul(
                        ps,
                        lhsT=pt3[:, k, mp * 128:(mp + 1) * 128],
                        rhs=xt3[:, k, :],
                        start=(k == 0),
                        stop=(k == KD - 1),
                    )
                # f32 -> fp8 with x4 pre-scale, on DVE: keeps the cast out of
                # the ScalarE queue where it would sit behind the previous
                # job's exp stream and delay this job's stage B
                nc.vector.tensor_scalar_mul(hidT3[:, mp, :], ps, HSCALE)

            # ---- Stage A2: token-major hidden (bf16) + target-logit dot ----
            nlts = []
            for ti in range(nti):
                ht = hpool.tile([128, pd + 1], BF16, name=f"ht{tag}_{ti}")
                nc.vector.memset(ht[:, pd:pd + 1], 1.0)
                for c0 in range(0, pd, 512):
                    cw = min(512, pd - c0)
                    ps = psA.tile([128, cw], F32, name=f"psA2_{tag}_{ti}_{c0}",
                                  tag="psA")
                    for k in range(KD):
                        nc.tensor.matmul(
                            ps,
                            lhsT=xt3[:, k, ti * 128:(ti + 1) * 128],
                            rhs=pt3[:, k, c0:c0 + cw],
                            start=(k == 0),
                            stop=(k == KD - 1),
                        )
                    nc.vector.tensor_copy(ht[:, c0:c0 + cw], ps)
                wgt = const.tile([128, pd + 1], BF16, name=f"wgt{tag}_{ti}")
                nc.gpsimd.dma_start(wgt, jb["wg"][ti, :, :])
                # absorb the DMA wait on a TensorCopy so the TensorTensor
                # below needs at most one sem wait
                dm = tiny.tile([128, 1], BF16, name=f"dm{tag}_{ti}")
                nc.vector.tensor_copy(dm, wgt[:, 0:1])
                prod = spool.tile([128, pd + 1], F32, name=f"prod{tag}_{ti}",
                                  tag="prod")
                nlt = tiny.tile([128, 1], F32, name=f"nlt{tag}_{ti}")
                nc.vector.tensor_mul(prod, ht, wgt)
                nc.vector.reduce_sum(nlt, prod, axis=X, negate=True)
                nlts.append(nlt)

            # ---- Stage B: fp8 DoubleRow logits + exp + running bf16 sum ----
            # chunks processed in pairs sharing one 2-bank PSUM tile; the exp
            # runs once per 1024 columns, and the exp'd pair is ADDED into a
            # running [128, 1024] bf16 accumulator (tensor_add at 2x bf16
            # rate); one small reduce per token tile happens in the finals
            npair = nch // 2
            accs = [
                hpool.tile([128, 2 * CHUNK], BF16, name=f"acc{tag}_{ti}")
                for ti in range(nti)
            ]
            for cpair in range(npair):
                wt = wpool.tile([128, kp * CHUNK * 2], FP8, name=f"wt{tag}_{cpair}",
                                tag="wt")
                off = kp * CHUNK * 2 * cpair
                nc.gpsimd.dma_start(wt, jb["wT"][:, off:off + kp * CHUNK * 2])
                wt4 = wt.rearrange("p (h g j c) -> p h g j c", h=2, g=ng, j=2)
                for ti in range(nti):
                    ps = psB.tile([128, 2 * CHUNK], F32,
                                  name=f"psB_{tag}_{cpair}_{ti}", tag="psB")
                    for h in range(2):
                        for g in range(ng):
                            nc.tensor.matmul(
                                ps[:, h * CHUNK:(h + 1) * CHUNK],
                                lhsT=hidT4[:, g, :, ti * 128:(ti + 1) * 128],
                                rhs=wt4[:, h, g, :, :],
                                start=(g == 0),
                                stop=(g == ng - 1),
                                perf_mode=DR,
                            )
                    scr = spool.tile([128, 2 * CHUNK], BF16,
                                     name=f"scr{tag}_{cpair}_{ti}", tag="scr",
                                     bufs=3)
                    nc.scalar.activation(scr, ps, Exp, scale=EXP_SCALE)
                    if cpair == 0:
                        nc.vector.tensor_copy(accs[ti], scr)
                    else:
                        nc.vector.tensor_add(accs[ti], accs[ti], scr)

            # ---- Finals: nll = ln(sum exp - n_pad) - logit_t - bias ----
            npad_t = const.tile([128, 1], F32, name=f"npad{tag}")
            if jb["npad"] is None:
                nc.gpsimd.dma_start(npad_t, npX_d[:, :])
            else:
                nc.vector.memset(npad_t, jb["npad"])
            for ti in range(nti):
                S = tiny.tile([128, 1], F32, name=f"S{tag}_{ti}")
                nc.vector.reduce_sum(S, accs[ti], axis=X)
                lse = tiny.tile([128, 1], F32, name=f"lse{tag}_{ti}")
                nc.scalar.activation(lse, S, Ln, bias=npad_t)
                nllt = tiny.tile([128, 1], F32, name=f"nllt{tag}_{ti}")
                nc.scalar.add(nllt, lse, nlts[ti])
                gt = t0 // 128 + ti
                nc.gpsimd.dma_start(out_d[gt:gt + 1, :], nllt)

    nc.finalize()
    return nc


def _get_graph():
    if "nc" not in _GRAPH_CACHE:
        _GRAPH_CACHE["nc"] = _build_graph()
    return _GRAPH_CACHE["nc"]


def _pack_pT(p):
    """[pd, D] f32 -> [128, KD, pd] bf16 (d-major partition tiles)."""
    pt = p.T.reshape(KD, 128, p.shape[0]).transpose(1, 0, 2)
    return np.ascontiguousarray(pt).astype(bf16)


def _pack_wT(w, kp, cpad):
    """[C, pd] f32 -> [128, kp*cpad] fp8 (x64), chunk-contiguous blocks."""
    pd, C = w.shape[1], w.shape[0]
    wk = w.T.reshape(kp, 128, C) * np.float32(WSCALE)
    wk = np.concatenate([wk, np.zeros((kp, 128, cpad - C), np.float32)], axis=2)
    blocks = [
        wk[:, :, off:off + CHUNK].transpose(1, 0, 2).reshape(128, -1)
        for off in range(0, cpad, CHUNK)
    ]
    return np.ascontiguousarray(np.concatenate(blocks, axis=1)).astype(fp8)


def _pack_shared(ps, ws):
    """Core-independent packed params (broadcast to every core)."""
    pT_host = [_pack_pT(ps[i]) for i in range(3)]
    wT_host = [_pack_wT(ws[i], KP[i], CPAD[i]) for i in range(3)]
    # X-pipeline variants: cluster 1 native (pd 512); cluster 2 zero-padded
    # from pd 256 to 512 (exact)
    p2pad = np.concatenate([ps[2], np.zeros((XPD - PD[2], D), np.float32)], axis=0)
    w2pad = np.concatenate(
        [ws[2], np.zeros((CSIZE[2], XPD - PD[2]), np.float32)], axis=1
    )
    pX_var = {1: _pack_pT(ps[1]), 2: _pack_pT(p2pad)}
    wX_var = {1: wT_host[1], 2: _pack_wT(w2pad, XKP, XCPAD)}
    return pT_host, wT_host, pX_var, wX_var


def kernel(x, target, p0, w0, b0, p1, w1, b1, p2, w2, b2):
    x = np.asarray(x, dtype=np.float32)
    tgt = np.asarray(target).astype(np.int64)
    ps = [np.asarray(p, np.float32) for p in (p0, p1, p2)]
    ws = [np.asarray(w, np.float32) for w in (w0, w1, w2)]
    bs = [np.asarray(b, np.float32) for b in (b0, b1, b2)]
    N = x.shape[0]

    cid = (tgt >= CUT[1]).astype(np.int32) + (tgt >= CUT[2]).astype(np.int32)
    perm = np.argsort(cid, kind="stable")
    segs = [perm[cid[perm] == i] for i in range(3)]

    # c0: even split over cores (capacity 256 each). c1/c2: exactly 384 per
    # core, remainder routed to the per-core X tiles.
    core_idx = [[None] * NCORES for _ in range(3)]
    core_idx[0] = np.array_split(segs[0], NCORES)
    for j in range(NCORES):
        if len(core_idx[0][j]) > CAP[0]:
            raise RuntimeError("cluster 0 capacity exceeded")
    xassign = []  # per X tile: (cluster, token ids)
    for i in (1, 2):
        seg = segs[i]
        main = CAP[i] * NCORES
        if len(seg) < main:
            # underflow: pad-split evenly, no X overflow for this cluster
            core_idx[i] = np.array_split(seg, NCORES)
        else:
            for j in range(NCORES):
                core_idx[i][j] = seg[j * CAP[i]:(j + 1) * CAP[i]]
            rem = seg[main:]
            for off in range(0, len(rem), XCAP):
                xassign.append((i, rem[off:off + XCAP]))
    if len(xassign) > NCORES:
        raise RuntimeError(f"X-tile capacity exceeded: {len(xassign)} > {NCORES}")
    while len(xassign) < NCORES:
        xassign.append((1, np.array([], dtype=np.int64)))

    pT_host, wT_host, pX_var, wX_var = _pack_shared(ps, ws)
    npX_var = {
        xc: np.full((128, 1), float(-NPAD[xc]), np.float32) for xc in (1, 2)
    }

    in_maps = []
    for j in range(NCORES):
        Xp = np.zeros((TOT, D), np.float32)
        m = {}
        for i in range(3):
            tk = core_idx[i][j]
            l = len(tk)
            Xp[SLOT0[i]:SLOT0[i] + l] = x[tk]
            wg = np.zeros((TILES[i] * 128, PD[i] + 1), np.float32)
            if l:
                local_t = (tgt[tk] - CUT[i]).astype(np.int64)
                wg[:l, :PD[i]] = ws[i][local_t]
                wg[:l, PD[i]] = bs[i][local_t]
            m[f"wg{i}"] = np.ascontiguousarray(
                wg.reshape(TILES[i], 128, PD[i] + 1)
            ).astype(bf16)
            m[f"pT{i}"] = pT_host[i]
            m[f"wT{i}"] = wT_host[i]
        xc, xtk = xassign[j]
        lx = len(xtk)
        if lx:
            Xp[XSLOT:XSLOT + lx] = x[xtk]
        wgx = np.zeros((128, XPD + 1), np.float32)
        if lx:
            local_t = (tgt[xtk] - CUT[xc]).astype(np.int64)
            wgx[:lx, :PD[xc]] = ws[xc][local_t]
            wgx[:lx, XPD] = bs[xc][local_t]
        m["wgX"] = np.ascontiguousarray(wgx.reshape(1, 128, XPD + 1)).astype(bf16)
        m["pX"] = pX_var[xc]
        m["wX"] = wX_var[xc]
        m["npX"] = npX_var[xc]
        xt = Xp.T.reshape(KD, 128, TOT).transpose(1, 0, 2)  # [128, KD, TOT]
        m["xT"] = np.ascontiguousarray(xt).astype(bf16)
        in_maps.append(m)

    nc = _get_graph()
    res = run_bass_kernel_spmd(nc, in_maps, core_ids=list(range(NCORES)))
    _GRAPH_CACHE["last_results"] = res  # for external profiling harnesses

    nll = np.zeros((N,), np.float32)
    for j in range(NCORES):
        flat = np.asarray(res.results[j]["out"], np.float32).reshape(TOT)
        for i in range(3):
            tk = core_idx[i][j]
            nll[tk] = flat[SLOT0[i]:SLOT0[i] + len(tk)]
        xc, xtk = xassign[j]
        if len(xtk):
            nll[xtk] = flat[XSLOT:XSLOT + len(xtk)]
    loss = np.float32(nll.sum(dtype=np.float32))
    return loss, nll


# revision 55
# speedup vs baseline: 1.0674x; 1.0074x over previous
# Adaptive softmax (3-cluster) on 8 TRN2 NeuronCores.
#
# Strategy (moe_routing): each token only needs its own cluster's pipeline.
# Host-side we sort tokens by cluster (pure data movement), shard each
# cluster's token segment evenly over the 8 cores, and pad each per-core
# segment to a static capacity so the Bass graph stays shape-static.
# Per core, per cluster c with nt tokens / pd proj dim / C classes:
#   hiddenT[pd, nt] = p_c @ x_shard^T            (PE, bf16)
#   hidden  [nt, pd]                             (PE, bf16; token-major)
#   logits  [nt, C] = hidden @ w_c^T             (PE, fp8 DoubleRow, chunked)
#   sumexp  [nt]    = sum_C exp(logits)          (ScalarE exp -> DVE 2x adds)
#   logit_t [nt]    = rowdot(hidden, w_c[tgt]) + b_c[tgt]    (DVE, bf16)
#   nll     [nt]    = ln(sumexp) - logit_t
# To cut token padding, c1/c2 are capped at 3 tiles (384) per core and the
# overflow tokens go to one generic "X tile" per core: a pd=512 /
# 20480-class pipeline whose projection/classifier weights are per-core
# inputs -- either cluster 1's natively or cluster 2's zero-padded from
# pd 256 to 512 (numerically exact), so the SPMD graph stays identical.
# The big logits GEMM runs in fp8e4m3 DoubleRow (2 rows/PE cell = 2x MACs).
# fp8 range handling: weights are pre-scaled x64 and hidden x4 (powers of
# two, lossless), and the exp activation rescales with scale=1/256.
# No max-subtraction is needed: |logits| <~ 4 for this problem's scales.
# Classes are padded to uniform 512-wide chunks with zero weights; each
# padded class contributes exactly exp(0)=1, corrected via the Ln pre-bias.
# The target-row weights w_c[tgt] are gathered on host (data movement only).
# No collectives: pure data parallelism; host gathers/unpermutes/sums.

import numpy as np
import ml_dtypes
from contextlib import ExitStack

import concourse.bass as bass
import concourse.bacc as bacc
import concourse.mybir as mybir
import concourse.tile as tile
from concourse.bass_utils import run_bass_kernel_spmd

BF16 = mybir.dt.bfloat16
FP8 = mybir.dt.float8e4
F32 = mybir.dt.float32
bf16 = ml_dtypes.bfloat16
fp8 = ml_dtypes.float8_e4m3

VOCAB = 50257
D = 1024           # input dim
KD = D // 128      # k-tiles over input dim
CUT = [0, 10000, 30000, VOCAB]
PD = [1024, 512, 256]            # per-cluster projection dims
KP = [p // 128 for p in PD]      # k-tiles over proj dim
CSIZE = [CUT[i + 1] - CUT[i] for i in range(3)]
NCORES = 8
CAP = [256, 384, 384]            # per-core token capacity per cluster
TILES = [c // 128 for c in CAP]
SLOT0 = [0, 256, 640]            # slot offset of each cluster's segment
XCAP = 128                       # one generic overflow tile per core
XSLOT = 1024
XPD = 512                        # X pipeline covers pd<=512 (c1 or c2-padded)
XKP = XPD // 128
TOT = sum(CAP) + XCAP            # 1152 padded tokens per core
NT = TOT // 128                  # 9 token tiles per core
CHUNK = 512                      # class chunk (one PSUM bank of f32)

HSCALE = 4.0                     # hidden fp8 pre-scale (power of 2)
WSCALE = 64.0                    # weight fp8 pre-scale (power of 2)
EXP_SCALE = 1.0 / (HSCALE * WSCALE)

# class padding to uniform chunks
CPAD = [-(-C // CHUNK) * CHUNK for C in CSIZE]
NPAD = [CPAD[i] - CSIZE[i] for i in range(3)]
CHUNKS = [[(off, CHUNK) for off in range(0, CPAD[i], CHUNK)] for i in range(3)]
assert CPAD[1] == CPAD[2] == 20480  # X pipeline class width
XCPAD = CPAD[1]
XF = XKP * XCPAD

# cluster processing order: smallest input first (shrinks the pre-matmul
# head); X between c1 and c0; c0 last (smallest tail)
ORDER = [2, 1, 0]

_GRAPH_CACHE = {}


def _build_graph():
    # Bacc (not plain Bass): its compile() pass splits semaphore waits into
    # event-semaphore carriers, satisfying TRN2's 1-wait-per-instruction limit.
    nc = bacc.Bacc(trn_type="TRN2", target_bir_lowering=False)

    xT_d = nc.dram_tensor("xT", [128, KD, TOT], BF16, kind="ExternalInput")
    pT_d = [
        nc.dram_tensor(f"pT{i}", [128, KD, PD[i]], BF16, kind="ExternalInput")
        for i in range(3)
    ]
    F = [KP[i] * CPAD[i] for i in range(3)]
    wT_d = [
        nc.dram_tensor(f"wT{i}", [128, F[i]], FP8, kind="ExternalInput")
        for i in range(3)
    ]
    # wg carries the gathered target-row weights plus one bias column
    wg_d = [
        nc.dram_tensor(f"wg{i}", [TILES[i], 128, PD[i] + 1], BF16, kind="ExternalInput")
        for i in range(3)
    ]
    # X-tile (overflow) pipeline inputs: per-core cluster choice baked by host
    pX_d = nc.dram_tensor("pX", [128, KD, XPD], BF16, kind="ExternalInput")
    wX_d = nc.dram_tensor("wX", [128, XF], FP8, kind="ExternalInput")
    wgX_d = nc.dram_tensor("wgX", [1, 128, XPD + 1], BF16, kind="ExternalInput")
    npX_d = nc.dram_tensor("npX", [128, 1], F32, kind="ExternalInput")
    out_d = nc.dram_tensor("out", [NT, 128], F32, kind="ExternalOutput")

    Exp = mybir.ActivationFunctionType.Exp
    Ln = mybir.ActivationFunctionType.Ln
    X = mybir.AxisListType.X
    DR = mybir.MatmulPerfMode.DoubleRow

    # (tag, ntok, t0, kp, pd, nti, nch, pT, wT, wg, npad_const)
    jobs_order = []
    for i in ORDER:
        jobs_order.append(dict(
            tag=f"{i}", ntok=CAP[i], t0=SLOT0[i], kp=KP[i], pd=PD[i],
            nti=TILES[i], nch=len(CHUNKS[i]), pT=pT_d[i], wT=wT_d[i],
            wg=wg_d[i], npad=float(-NPAD[i]),
        ))
    xjob = dict(
        tag="x", ntok=XCAP, t0=XSLOT, kp=XKP, pd=XPD, nti=1,
        nch=len(CHUNKS[1]), pT=pX_d, wT=wX_d, wg=wgX_d, npad=None,
    )
    jobs_order.append(xjob)  # c2, c1, c0, X (single-tile X job = short tail)

    with ExitStack() as ctx:
        tc = ctx.enter_context(tile.TileContext(nc))
        const = ctx.enter_context(tc.tile_pool(name="const", bufs=1))
        wpool = ctx.enter_context(tc.tile_pool(name="wpool", bufs=8))
        hpool = ctx.enter_context(tc.tile_pool(name="hpool", bufs=1))
        spool = ctx.enter_context(tc.tile_pool(name="spool", bufs=2))
        tiny = ctx.enter_context(tc.tile_pool(name="tiny", bufs=1))
        psA = ctx.enter_context(tc.tile_pool(name="psA", bufs=2, space="PSUM"))
        psB = ctx.enter_context(tc.tile_pool(name="psB", bufs=3, space="PSUM"))

        # input DMAs for all jobs up front (processing order first) so a
        # job's pt/xt land before the previous job's weight stream
        # monopolizes the DMA lanes
        for jb in jobs_order:
            pt = const.tile([128, KD * jb["pd"]], BF16, name=f"pt{jb['tag']}")
            nc.gpsimd.dma_start(pt, jb["pT"][:, :, :])
            xt = const.tile([128, KD * jb["ntok"]], BF16, name=f"xt{jb['tag']}")
            nc.gpsimd.dma_start(xt, xT_d[:, :, jb["t0"]:jb["t0"] + jb["ntok"]])
            jb["pt3"] = pt.rearrange("p (k m) -> p k m", k=KD)
            jb["xt3"] = xt.rearrange("p (k t) -> p k t", k=KD)

        for jb in jobs_order:
            tag, ntok, t0 = jb["tag"], jb["ntok"], jb["t0"]
            kp, pd, nti, nch = jb["kp"], jb["pd"], jb["nti"], jb["nch"]
            ng = kp // 2  # DoubleRow contraction groups (K=256 each)
            pt3, xt3 = jb["pt3"], jb["xt3"]

            # ---- Stage A1: hiddenT [pd, ntok] as fp8 (x4), pd-major ----
            hidT = hpool.tile([128, kp * ntok], FP8, name=f"hidT{tag}")
            hidT3 = hidT.rearrange("p (k t) -> p k t", k=kp)
            hidT4 = hidT.rearrange("p (g j t) -> p g j t", g=ng, j=2)
            for mp in range(kp):
                ps = psA.tile([128, ntok], F32, name=f"psA1_{tag}_{mp}", tag="psA")
                for k in range(KD):
                    nc.tensor.matmul(
                        ps,
                        lhsT=pt3[:, k, mp * 128:(mp + 1) * 128],
                        rhs=xt3[:, k, :],
                        start=(k == 0),
                        stop=(k == KD - 1),
                    )
                # f32 -> fp8 with x4 pre-scale, on DVE: keeps the cast out of
                # the ScalarE queue where it would sit behind the previous
                # job's exp stream and delay this job's stage B
                nc.vector.tensor_scalar_mul(hidT3[:, mp, :], ps, HSCALE)

            # ---- Stage A2: token-major hidden (bf16) + target-logit dot ----
            nlts = []
            for ti in range(nti):
                ht = hpool.tile([128, pd + 1], BF16, name=f"ht{tag}_{ti}")
                nc.vector.memset(ht[:, pd:pd + 1], 1.0)
                for c0 in range(0, pd, 512):
                    cw = min(512, pd - c0)
                    ps = psA.tile([128, cw], F32, name=f"psA2_{tag}_{ti}_{c0}",
                                  tag="psA")
                    for k in range(KD):
                        nc.tensor.matmul(
                            ps,
                            lhsT=xt3[:, k, ti * 128:(ti + 1) * 128],
                            rhs=pt3[:, k, c0:c0 + cw],
                            start=(k == 0),
                            stop=(k == KD - 1),
                        )
                    nc.vector.tensor_copy(ht[:, c0:c0 + cw], ps)
                wgt = const.tile([128, pd + 1], BF16, name=f"wgt{tag}_{ti}")
                nc.gpsimd.dma_start(wgt, jb["wg"][ti, :, :])
                # absorb the DMA wait on a TensorCopy so the TensorTensor
                # below needs at most one sem wait
                dm = tiny.tile([128, 1], BF16, name=f"dm{tag}_{ti}")
                nc.vector.tensor_copy(dm, wgt[:, 0:1])
                prod = spool.tile([128, pd + 1], F32, name=f"prod{tag}_{ti}",
                                  tag="prod")
                nlt = tiny.tile([128, 1], F32, name=f"nlt{tag}_{ti}")
                nc.vector.tensor_mul(prod, ht, wgt)
                nc.vector.reduce_sum(nlt, prod, axis=X, negate=True)
                nlts.append(nlt)

            # ---- Stage B: fp8 DoubleRow logits + exp + running bf16 sum ----
            # chunks processed in pairs sharing one 2-bank PSUM tile; the exp
            # runs once per 1024 columns, and the exp'd pair is ADDED into a
            # running [128, 1024] bf16 accumulator (tensor_add at 2x bf16
            # rate); one small reduce per token tile happens in the finals
            npair = nch // 2
            accs = [
                hpool.tile([128, 2 * CHUNK], BF16, name=f"acc{tag}_{ti}")
                for ti in range(nti)
            ]
            for cpair in range(npair):
                wt = wpool.tile([128, kp * CHUNK * 2], FP8, name=f"wt{tag}_{cpair}",
                                tag="wt")
                off = kp * CHUNK * 2 * cpair
                nc.gpsimd.dma_start(wt, jb["wT"][:, off:off + kp * CHUNK * 2])
                wt4 = wt.rearrange("p (h g j c) -> p h g j c", h=2, g=ng, j=2)
                for ti in range(nti):
                    ps = psB.tile([128, 2 * CHUNK], F32,
                                  name=f"psB_{tag}_{cpair}_{ti}", tag="psB")
                    for h in range(2):
                        for g in range(ng):
                            nc.tensor.matmul(
                                ps[:, h * CHUNK:(h + 1) * CHUNK],
                                lhsT=hidT4[:, g, :, ti * 128:(ti + 1) * 128],
                                rhs=wt4[:, h, g, :, :],
                                start=(g == 0),
                                stop=(g == ng - 1),
                                perf_mode=DR,
                            )
                    scr = spool.tile([128, 2 * CHUNK], BF16,
                                     name=f"scr{tag}_{cpair}_{ti}", tag="scr",
                                     bufs=3)
                    nc.scalar.activation(scr, ps, Exp, scale=EXP_SCALE)
                    if cpair == 0:
                        nc.vector.tensor_copy(accs[ti], scr)
                    else:
                        nc.vector.tensor_add(accs[ti], accs[ti], scr)

            # ---- Finals: nll = ln(sum exp - n_pad) - logit_t - bias ----
            npad_t = const.tile([128, 1], F32, name=f"npad{tag}")
            if jb["npad"] is None:
                nc.gpsimd.dma_start(npad_t, npX_d[:, :])
            else:
                nc.vector.memset(npad_t, jb["npad"])
            for ti in range(nti):
                S = tiny.tile([128, 1], F32, name=f"S{tag}_{ti}")
                nc.vector.reduce_sum(S, accs[ti], axis=X)
                lse = tiny.tile([128, 1], F32, name=f"lse{tag}_{ti}")
                nc.scalar.activation(lse, S, Ln, bias=npad_t)
                nllt = tiny.tile([128, 1], F32, name=f"nllt{tag}_{ti}")
                nc.scalar.add(nllt, lse, nlts[ti])
                gt = t0 // 128 + ti
                nc.gpsimd.dma_start(out_d[gt:gt + 1, :], nllt)

    nc.finalize()
    return nc


def _get_graph():
    if "nc" not in _GRAPH_CACHE:
        _GRAPH_CACHE["nc"] = _build_graph()
    return _GRAPH_CACHE["nc"]


def _pack_pT(p):
    """[pd, D] f32 -> [128, KD, pd] bf16 (d-major partition tiles)."""
    pt = p.T.reshape(KD, 128, p.shape[0]).transpose(1, 0, 2)
    return np.ascontiguousarray(pt).astype(bf16)


def _pack_wT(w, kp, cpad):
    """[C, pd] f32 -> [128, kp*cpad] fp8 (x64), chunk-contiguous blocks."""
    pd, C = w.shape[1], w.shape[0]
    wk = w.T.reshape(kp, 128, C) * np.float32(WSCALE)
    wk = np.concatenate([wk, np.zeros((kp, 128, cpad - C), np.float32)], axis=2)
    blocks = [
        wk[:, :, off:off + CHUNK].transpose(1, 0, 2).reshape(128, -1)
        for off in range(0, cpad, CHUNK)
    ]
    return np.ascontiguousarray(np.concatenate(blocks, axis=1)).astype(fp8)


def _pack_shared(ps, ws):
    """Core-independent packed params (broadcast to every core)."""
    pT_host = [_pack_pT(ps[i]) for i in range(3)]
    wT_host = [_pack_wT(ws[i], KP[i], CPAD[i]) for i in range(3)]
    # X-pipeline variants: cluster 1 native (pd 512); cluster 2 zero-padded
    # from pd 256 to 512 (exact)
    p2pad = np.concatenate([ps[2], np.zeros((XPD - PD[2], D), np.float32)], axis=0)
    w2pad = np.concatenate(
        [ws[2], np.zeros((CSIZE[2], XPD - PD[2]), np.float32)], axis=1
    )
    pX_var = {1: _pack_pT(ps[1]), 2: _pack_pT(p2pad)}
    wX_var = {1: wT_host[1], 2: _pack_wT(w2pad, XKP, XCPAD)}
    return pT_host, wT_host, pX_var, wX_var


def kernel(x, target, p0, w0, b0, p1, w1, b1, p2, w2, b2):
    x = np.asarray(x, dtype=np.float32)
    tgt = np.asarray(target).astype(np.int64)
    ps = [np.asarray(p, np.float32) for p in (p0, p1, p2)]
    ws = [np.asarray(w, np.float32) for w in (w0, w1, w2)]
    bs = [np.asarray(b, np.float32) for b in (b0, b1, b2)]
    N = x.shape[0]

    cid = (tgt >= CUT[1]).astype(np.int32) + (tgt >= CUT[2]).astype(np.int32)
    perm = np.argsort(cid, kind="stable")
    segs = [perm[cid[perm] == i] for i in range(3)]

    # c0: even split over cores (capacity 256 each). c1/c2: exactly 384 per
    # core, remainder routed to the per-core X tiles.
    core_idx = [[None] * NCORES for _ in range(3)]
    core_idx[0] = np.array_split(segs[0], NCORES)
    for j in range(NCORES):
        if len(core_idx[0][j]) > CAP[0]:
            raise RuntimeError("cluster 0 capacity exceeded")
    xassign = []  # per X tile: (cluster, token ids)
    for i in (1, 2):
        seg = segs[i]
        main = CAP[i] * NCORES
        if len(seg) < main:
            # underflow: pad-split evenly, no X overflow for this cluster
            core_idx[i] = np.array_split(seg, NCORES)
        else:
            for j in range(NCORES):
                core_idx[i][j] = seg[j * CAP[i]:(j + 1) * CAP[i]]
            rem = seg[main:]
            for off in range(0, len(rem), XCAP):
                xassign.append((i, rem[off:off + XCAP]))
    if len(xassign) > NCORES:
        raise RuntimeError(f"X-tile capacity exceeded: {len(xassign)} > {NCORES}")
    while len(xassign) < NCORES:
        xassign.append((1, np.array([], dtype=np.int64)))

    pT_host, wT_host, pX_var, wX_var = _pack_shared(ps, ws)
    npX_var = {
        xc: np.full((128, 1), float(-NPAD[xc]), np.float32) for xc in (1, 2)
    }

    in_maps = []
    for j in range(NCORES):
        Xp = np.zeros((TOT, D), np.float32)
        m = {}
        for i in range(3):
            tk = core_idx[i][j]
            l = len(tk)
            Xp[SLOT0[i]:SLOT0[i] + l] = x[tk]
            wg = np.zeros((TILES[i] * 128, PD[i] + 1), np.float32)
            if l:
                local_t = (tgt[tk] - CUT[i]).astype(np.int64)
                wg[:l, :PD[i]] = ws[i][local_t]
                wg[:l, PD[i]] = bs[i][local_t]
            m[f"wg{i}"] = np.ascontiguousarray(
                wg.reshape(TILES[i], 128, PD[i] + 1)
            ).astype(bf16)
            m[f"pT{i}"] = pT_host[i]
            m[f"wT{i}"] = wT_host[i]
        xc, xtk = xassign[j]
        lx = len(xtk)
        if lx:
            Xp[XSLOT:XSLOT + lx] = x[xtk]
        wgx = np.zeros((128, XPD + 1), np.float32)
        if lx:
            local_t = (tgt[xtk] - CUT[xc]).astype(np.int64)
            wgx[:lx, :PD[xc]] = ws[xc][local_t]
            wgx[:lx, XPD] = bs[xc][local_t]
        m["wgX"] = np.ascontiguousarray(wgx.reshape(1, 128, XPD + 1)).astype(bf16)
        m["pX"] = pX_var[xc]
        m["wX"] = wX_var[xc]
        m["npX"] = npX_var[xc]
        xt = Xp.T.reshape(KD, 128, TOT).transpose(1, 0, 2)  # [128, KD, TOT]
        m["xT"] = np.ascontiguousarray(xt).astype(bf16)
        in_maps.append(m)

    nc = _get_graph()
    res = run_bass_kernel_spmd(nc, in_maps, core_ids=list(range(NCORES)))
    _GRAPH_CACHE["last_results"] = res  # for external profiling harnesses

    nll = np.zeros((N,), np.float32)
    for j in range(NCORES):
        flat = np.asarray(res.results[j]["out"], np.float32).reshape(TOT)
        for i in range(3):
            tk = core_idx[i][j]
            nll[tk] = flat[SLOT0[i]:SLOT0[i] + len(tk)]
        xc, xtk = xassign[j]
        if len(xtk):
            nll[xtk] = flat[XSLOT:XSLOT + len(xtk)]
    loss = np.float32(nll.sum(dtype=np.float32))
    return loss, nll
